# revision 1
# baseline (speedup 1.0000x reference)
"""Trainium2 Bass kernel for nn_Attention_4183298146960.

GQA causal attention layer: B=2, S=2048, HIDDEN=2048, 16 q heads / 4 kv heads,
head_dim=128, RoPE (interleaved pairs), causal softmax, output projection.

Sharding (8 cores, SPMD-uniform program):
  core c owns q heads {2c, 2c+1} and kv head c//2, for BOTH batches
  (tokens axis = [batch0 | batch1] = 4096).  QKV + RoPE + attention are
  fully local; the output projection needs all 16 heads' features, so the
  per-core attention outputs [256 feat, 4096 tok] are AllGathered (8-core
  mesh, chunked over 2 token slabs to overlap with attention), after which
  each core computes its 256 output columns of W_o.

Layouts on device (partition dim first):
  feature-major qT/kT [head_dim, tokens] for scores; token-major v
  [tokens, head_dim] for PV; scores computed transposed [k, q] so softmax
  needs no max-subtraction (scores are O(+-10), exp is fp32-safe) and the
  denominator is a ones-vector matmul; probabilities stay unnormalized
  until after PV (flash-attention style deferred normalization).
  RoPE head dims are permuted [even dims | odd dims] via host-side W row
  permutation so the rotation is a 64-partition swap (SBUF->SBUF DMA) plus
  elementwise DVE ops.
"""

import numpy as np
import ml_dtypes

import concourse.bass as bass
import concourse.mybir as mybir
import concourse.tile as tile
from concourse import bacc
from concourse.bass_utils import run_bass_kernel_spmd

BF16 = ml_dtypes.bfloat16
FP32 = np.float32

HEADS = 16
KV_HEADS = 4
HIDDEN = 2048
HD = 128
S = 2048
B = 2
T = B * S                      # 4096 token axis (both batches)
HT = HIDDEN // 128             # 16 hidden tiles
NQT = S // 512                 # 4 q-tiles of 512 per batch
SCALE = 1.0 / float(np.sqrt(HD))
RG8 = [[0, 1, 2, 3, 4, 5, 6, 7]]
NCHUNK = 2                     # AllGather chunks over the q-token axis

_COMPILED = None


def _build():
    dt = mybir.dt
    nc = bacc.Bacc("TRN2", target_bir_lowering=False, debug=False, num_devices=8)

    xT = nc.dram_tensor("xT", [128, HT, T], dt.bfloat16, kind="ExternalInput")
    wqk = nc.dram_tensor("wqk", [128, HT, 384], dt.bfloat16, kind="ExternalInput")
    wv = nc.dram_tensor("wv", [128, HT, 128], dt.bfloat16, kind="ExternalInput")
    wo = nc.dram_tensor("wo", [128, HT, 256], dt.bfloat16, kind="ExternalInput")
    cc = nc.dram_tensor("cc", [128, T], dt.float32, kind="ExternalInput")
    ss = nc.dram_tensor("ss", [128, T], dt.float32, kind="ExternalInput")
    msk = nc.dram_tensor("msk", [128, 4 * 512], dt.bfloat16, kind="ExternalInput")
    onec = nc.dram_tensor("onec", [128, 1], dt.bfloat16, kind="ExternalInput")
    oner = nc.dram_tensor("oner", [1, 128], dt.bfloat16, kind="ExternalInput")
    outT = nc.dram_tensor("outT", [256, T], dt.float32, kind="ExternalOutput")

    mult = mybir.AluOpType.mult
    add = mybir.AluOpType.add
    Exp = mybir.ActivationFunctionType.Exp

    with tile.TileContext(nc) as tc:
        with (
            tc.tile_pool(name="const", bufs=1) as constp,
            tc.tile_pool(name="dram", bufs=1, space="DRAM") as dram,
        ):
            wo_sb = constp.tile([128, HT, 256], dt.bfloat16)
            msk_sb = constp.tile([128, 4 * 512], dt.bfloat16)
            onec_sb = constp.tile([128, 1], dt.bfloat16)
            oner_sb = constp.tile([1, 128], dt.bfloat16)
            qcat = constp.tile([128, 2 * T], dt.bfloat16)   # 2 local q heads
            kT = constp.tile([128, T], dt.bfloat16)
            vsb = constp.tile([128, T], dt.bfloat16)        # token-major v tiles
            nc.sync.dma_start(wo_sb[:], wo[:])
            nc.sync.dma_start(msk_sb[:], msk[:])
            nc.sync.dma_start(onec_sb[:], onec[:])
            nc.sync.dma_start(oner_sb[:], oner[:])

            # ---------------- QKV + RoPE ----------------
            with (
                tc.tile_pool(name="qkvw", bufs=1) as qkvw,
                tc.tile_pool(name="xp", bufs=2) as xp,
                tc.tile_pool(name="rp", bufs=3) as rp,
                tc.tile_pool(name="qkps", bufs=3, space="PSUM") as qkps,
                tc.tile_pool(name="vps", bufs=2, space="PSUM") as vps,
            ):
                wqk_sb = qkvw.tile([128, HT, 384], dt.bfloat16)
                wv_sb = qkvw.tile([128, HT, 128], dt.bfloat16)
                cc_sb = qkvw.tile([128, T], dt.float32)
                ss_sb = qkvw.tile([128, T], dt.float32)
                nc.sync.dma_start(wqk_sb[:], wqk[:])
                nc.sync.dma_start(wv_sb[:], wv[:])
                nc.sync.dma_start(cc_sb[:], cc[:])
                nc.sync.dma_start(ss_sb[:], ss[:])

                for tt in range(T // 512):
                    x_sb = xp.tile([128, HT, 512], dt.bfloat16)
                    nc.sync.dma_start(x_sb[:], xT[:, :, tt * 512:(tt + 1) * 512])
                    tsl = bass.ts(tt, 512)
                    for ft in range(3):  # q0, q1, k (feature-major + rope)
                        ps = qkps.tile([128, 512], dt.float32)
                        for ht in range(HT):
                            nc.tensor.matmul(
                                ps[:],
                                lhsT=wqk_sb[:, ht, ft * 128:(ft + 1) * 128],
                                rhs=x_sb[:, ht, :],
                                start=(ht == 0),
                                stop=(ht == HT - 1),
                            )
                        sbq = rp.tile([128, 512], dt.float32)
                        nc.scalar.copy(sbq[:], ps[:])
                        tmp = rp.tile([128, 512], dt.float32)
                        nc.sync.dma_start(tmp[0:64, :], sbq[64:128, :])
                        nc.sync.dma_start(tmp[64:128, :], sbq[0:64, :])
                        qcc = rp.tile([128, 512], dt.float32)
                        nc.vector.tensor_tensor(qcc[:], sbq[:], cc_sb[:, tsl], mult)
                        qss = rp.tile([128, 512], dt.float32)
                        nc.vector.tensor_tensor(qss[:], tmp[:], ss_sb[:, tsl], mult)
                        if ft < 2:
                            dst = qcat[:, ft * T + tt * 512: ft * T + (tt + 1) * 512]
                        else:
                            dst = kT[:, tsl]
                        nc.vector.tensor_tensor(dst, qcc[:], qss[:], add)
                    for st in range(4):  # token-major v
                        psv = vps.tile([128, 128], dt.float32)
                        for ht in range(HT):
                            nc.tensor.matmul(
                                psv[:],
                                lhsT=x_sb[:, ht, st * 128:(st + 1) * 128],
                                rhs=wv_sb[:, ht, :],
                                start=(ht == 0),
                                stop=(ht == HT - 1),
                            )
                        t128 = tt * 4 + st
                        nc.scalar.copy(vsb[:, t128 * 128:(t128 + 1) * 128], psv[:])

            # ---------------- attention + AllGather + W_o ----------------
            qt_per_chunk = NQT // NCHUNK
            with (
                tc.tile_pool(name="spool", bufs=2, space="PSUM") as spool,
                tc.tile_pool(name="pvp", bufs=2, space="PSUM") as pvp,
                tc.tile_pool(name="denp", bufs=1, space="PSUM") as denp,
                tc.tile_pool(name="bcp", bufs=1, space="PSUM") as bcp,
                tc.tile_pool(name="wops", bufs=2, space="PSUM") as wops,
                tc.tile_pool(name="probs", bufs=4) as probs,
                tc.tile_pool(name="smallp", bufs=4) as smallp,
                tc.tile_pool(name="ap", bufs=3) as apool,
                tc.tile_pool(name="wosb", bufs=2) as wosb,
                tc.tile_pool(name="outp", bufs=2) as outp,
            ):
                for j in range(NCHUNK):
                    attn_chunk = dram.tile(
                        [256, 4 * qt_per_chunk * 512], dt.bfloat16, name=f"attnc{j}"
                    )
                    for qtl in range(qt_per_chunk):
                        qt = j * qt_per_chunk + qtl
                        for b in range(B):
                            col0 = (b * qt_per_chunk + qtl) * 512
                            for hl in range(2):
                                kts = 4 * qt + 4
                                ps_pv = pvp.tile([128, 512], dt.float32)
                                ps_den = denp.tile([1, 512], dt.float32)
                                for kt in range(kts):
                                    ps_s = spool.tile([128, 512], dt.float32)
                                    nc.tensor.matmul(
                                        ps_s[:],
                                        lhsT=kT[:, b * S + kt * 128: b * S + (kt + 1) * 128],
                                        rhs=qcat[:, hl * T + b * S + qt * 512: hl * T + b * S + (qt + 1) * 512],
                                        start=True,
                                        stop=True,
                                    )
                                    r = kt - 4 * qt
                                    prob = probs.tile([128, 512], dt.bfloat16)
                                    if r >= 0:
                                        stg = probs.tile([128, 512], dt.bfloat16, name="stg")
                                        nc.scalar.activation(stg[:], ps_s[:], Exp, scale=SCALE)
                                        nc.vector.tensor_tensor(
                                            prob[:], stg[:], msk_sb[:, r * 512:(r + 1) * 512], mult
                                        )
                                    else:
                                        nc.scalar.activation(prob[:], ps_s[:], Exp, scale=SCALE)
                                    nc.tensor.matmul(
                                        ps_den[:], lhsT=onec_sb[:], rhs=prob[:],
                                        start=(kt == 0), stop=(kt == kts - 1),
                                    )
                                    nc.tensor.matmul(
                                        ps_pv[:],
                                        lhsT=vsb[:, (b * HT + kt) * 128: (b * HT + kt + 1) * 128],
                                        rhs=prob[:],
                                        start=(kt == 0), stop=(kt == kts - 1),
                                    )
                                den_f = smallp.tile([1, 512], dt.float32)
                                nc.vector.reciprocal(den_f[:], ps_den[:])
                                den_b = smallp.tile([1, 512], dt.bfloat16)
                                nc.scalar.copy(den_b[:], den_f[:])
                                ps_bc = bcp.tile([128, 512], dt.float32)
                                nc.tensor.matmul(
                                    ps_bc[:], lhsT=oner_sb[:], rhs=den_b[:],
                                    start=True, stop=True,
                                )
                                bc_sb = smallp.tile([128, 512], dt.float32)
                                nc.scalar.copy(bc_sb[:], ps_bc[:])
                                attn_sb = apool.tile([128, 512], dt.bfloat16)
                                nc.vector.tensor_tensor(attn_sb[:], ps_pv[:], bc_sb[:], mult)
                                nc.sync.dma_start(
                                    attn_chunk[hl * 128:(hl + 1) * 128, col0:col0 + 512],
                                    attn_sb[:],
                                )
                    ag_out = dram.tile(
                        [HT, 128, 4 * qt_per_chunk * 512], dt.bfloat16,
                        addr_space="Shared", name=f"agout{j}",
                    )
                    nc.gpsimd.collective_compute(
                        "AllGather", mybir.AluOpType.bypass, replica_groups=RG8,
                        ins=[attn_chunk.opt()], outs=[ag_out.opt()],
                    )
                    # W_o for this chunk's tokens
                    for cb in range(2 * qt_per_chunk):
                        b = cb // qt_per_chunk
                        qt = j * qt_per_chunk + (cb % qt_per_chunk)
                        asb = wosb.tile([128, HT, 512], dt.bfloat16)
                        for dtt in range(HT):
                            nc.sync.dma_start(
                                asb[:, dtt, :], ag_out[dtt, :, cb * 512:(cb + 1) * 512]
                            )
                        for ct in range(2):
                            ps_o = wops.tile([128, 512], dt.float32)
                            for dtt in range(HT):
                                nc.tensor.matmul(
                                    ps_o[:],
                                    lhsT=wo_sb[:, dtt, ct * 128:(ct + 1) * 128],
                                    rhs=asb[:, dtt, :],
                                    start=(dtt == 0), stop=(dtt == HT - 1),
                                )
                            o_sb = outp.tile([128, 512], dt.float32)
                            nc.scalar.copy(o_sb[:], ps_o[:])
                            nc.sync.dma_start(
                                outT[ct * 128:(ct + 1) * 128, b * S + qt * 512: b * S + (qt + 1) * 512],
                                o_sb[:],
                            )
    nc.compile()
    return nc


# host-side input prep ------------------------------------------------------

_PERM = np.concatenate([np.arange(0, HD, 2), np.arange(1, HD, 2)])


def _rope_tables():
    freq = 1.0 / (10000.0 ** (np.arange(0, HD, 2, dtype=np.float64) / HD))
    pos = np.arange(S, dtype=np.float64)
    ang = np.outer(pos, freq)                       # [S, 64]
    cos = np.cos(ang).T.astype(np.float32)          # [64, S]
    sin = np.sin(ang).T.astype(np.float32)
    cc1 = np.concatenate([cos, cos], 0)             # [128, S]
    ss1 = np.concatenate([-sin, sin], 0)            # [128, S]
    return np.tile(cc1, (1, B)), np.tile(ss1, (1, B))   # [128, 4096]


def _prep_inputs(x, W_qkv, W_o):
    x = np.asarray(x, dtype=np.float32)
    W_qkv = np.asarray(W_qkv, dtype=np.float32)
    W_o = np.asarray(W_o, dtype=np.float32)

    xx = np.concatenate([x[0], x[1]], axis=0)       # [4096, 2048]
    xTd = np.ascontiguousarray(
        xx.T.reshape(HT, 128, T).transpose(1, 0, 2)
    ).astype(BF16)                                   # [128, HT, 4096]

    cc, ss = _rope_tables()

    mask = np.zeros((128, 4 * 512), dtype=np.float32)
    ii = np.arange(128)[:, None]
    jj = np.arange(512)[None, :]
    for r in range(4):
        mask[:, r * 512:(r + 1) * 512] = (jj >= ii + 128 * r)
    mask = mask.astype(BF16)

    onec = np.ones((128, 1), dtype=np.float32).astype(BF16)
    oner = np.ones((1, 128), dtype=np.float32).astype(BF16)

    in_maps = []
    for c in range(8):
        kh = c // 2
        qr = W_qkv[256 * c: 256 * (c + 1)]           # rows of q heads 2c,2c+1
        qr = qr.reshape(2, HD, HIDDEN)[:, _PERM, :].reshape(256, HIDDEN)
        kr = W_qkv[HIDDEN + 128 * kh: HIDDEN + 128 * (kh + 1)][_PERM, :]
        vr = W_qkv[HIDDEN + 512 + 128 * kh: HIDDEN + 512 + 128 * (kh + 1)]
        wqkT = np.ascontiguousarray(
            np.concatenate([qr, kr], 0).T.reshape(HT, 128, 384).transpose(1, 0, 2)
        ).astype(BF16)                               # [128, HT, 384]
        wvT = np.ascontiguousarray(
            vr.T.reshape(HT, 128, 128).transpose(1, 0, 2)
        ).astype(BF16)
        woT = np.ascontiguousarray(
            W_o[256 * c: 256 * (c + 1)].T.reshape(HT, 128, 256).transpose(1, 0, 2)
        ).astype(BF16)
        in_maps.append({
            "xT": xTd, "wqk": wqkT, "wv": wvT, "wo": woT,
            "cc": cc, "ss": ss, "msk": mask, "onec": onec, "oner": oner,
        })
    return in_maps


def kernel(x, W_qkv, W_o):
    global _COMPILED
    if _COMPILED is None:
        _COMPILED = _build()
    nc = _COMPILED
    in_maps = _prep_inputs(x, W_qkv, W_o)
    res = run_bass_kernel_spmd(nc, in_maps, list(range(8)))
    out = np.empty((B, S, HIDDEN), dtype=np.float32)
    for c in range(8):
        oT = res.results[c]["outT"]                  # [256, 4096]
        out[:, :, 256 * c: 256 * (c + 1)] = oT.reshape(256, B, S).transpose(1, 2, 0)
    return out


# revision 5
# speedup vs baseline: 1.1793x; 1.1793x over previous
"""Trainium2 Bass kernel for nn_Attention_4183298146960.

GQA causal attention layer: B=2, S=2048, HIDDEN=2048, 16 q heads / 4 kv heads,
head_dim=128, RoPE (interleaved pairs), causal softmax, output projection.

Sharding (8 cores, SPMD-uniform program):
  core c owns q heads {2c, 2c+1} and kv head c//2, for BOTH batches
  (tokens axis = [batch0 | batch1] = 4096).  QKV + RoPE + attention are
  fully local; the output projection needs all 16 heads' features, so the
  per-core attention outputs [256 feat, 4096 tok] are AllGathered (8-core
  mesh, chunked over q-tile slabs processed in reverse causal order so the
  gathers overlap attention compute), after which each core computes its
  256 output columns of W_o.

Layouts on device (partition dim first):
  feature-major qT/kT [head_dim, tokens] for scores; token-major v
  [tokens, head_dim] for PV; scores computed transposed [k, q] so softmax
  needs no max-subtraction (scores are O(+-10), exp is fp32-safe) and the
  denominator is a ones-matrix matmul producing the broadcast denominator
  directly; probabilities stay unnormalized until after PV.
  RoPE head dims are permuted [even | odd] via host-side W row permutation
  so the rotation is a 64-partition swap (SBUF->SBUF DMA) + DVE ops.
"""

import numpy as np
import ml_dtypes

import concourse.bass as bass
import concourse.mybir as mybir
import concourse.tile as tile
from concourse import bacc
from concourse.bass_utils import run_bass_kernel_spmd

BF16 = ml_dtypes.bfloat16

HEADS = 16
KV_HEADS = 4
HIDDEN = 2048
HD = 128
S = 2048
B = 2
T = B * S                      # 4096 token axis (both batches)
HT = HIDDEN // 128             # 16 hidden tiles
NQT = S // 512                 # 4 q-tiles of 512 per batch
SCALE = 1.0 / float(np.sqrt(HD))
RG8 = [[0, 1, 2, 3, 4, 5, 6, 7]]
QT_ORDER = [3, 2, 1, 0]        # chunk order: biggest (most causal kt) first

_COMPILED = None


def _build():
    dt = mybir.dt
    nc = bacc.Bacc("TRN2", target_bir_lowering=False, debug=False, num_devices=8)

    xT = nc.dram_tensor("xT", [128, HT, T], dt.bfloat16, kind="ExternalInput")
    wqk = nc.dram_tensor("wqk", [128, HT, 384], dt.bfloat16, kind="ExternalInput")
    wv = nc.dram_tensor("wv", [128, HT, 128], dt.bfloat16, kind="ExternalInput")
    wo = nc.dram_tensor("wo", [128, HT, 256], dt.bfloat16, kind="ExternalInput")
    cc = nc.dram_tensor("cc", [128, T], dt.float32, kind="ExternalInput")
    ss = nc.dram_tensor("ss", [128, T], dt.float32, kind="ExternalInput")
    msk = nc.dram_tensor("msk", [128, 4 * 512], dt.bfloat16, kind="ExternalInput")
    ones128 = nc.dram_tensor("ones128", [128, 128], dt.bfloat16, kind="ExternalInput")
    outT = nc.dram_tensor("outT", [256, T], dt.float32, kind="ExternalOutput")

    mult = mybir.AluOpType.mult
    add = mybir.AluOpType.add
    divide = mybir.AluOpType.divide
    Exp = mybir.ActivationFunctionType.Exp

    with tile.TileContext(nc) as tc:
        with (
            tc.tile_pool(name="const", bufs=1) as constp,
            tc.tile_pool(name="dram", bufs=1, space="DRAM") as dram,
        ):
            qcat = constp.tile([128, 2 * T], dt.bfloat16)   # 2 local q heads
            kT = constp.tile([128, T], dt.bfloat16)
            vsb = constp.tile([128, T], dt.bfloat16)        # token-major v tiles
            wo_sb = constp.tile([128, HT, 256], dt.bfloat16)
            msk_sb = constp.tile([128, 4 * 512], dt.bfloat16)
            ones_sb = constp.tile([128, 128], dt.bfloat16)

            # ---------------- QKV + RoPE ----------------
            with (
                tc.tile_pool(name="qkvw", bufs=1) as qkvw,
                tc.tile_pool(name="xp", bufs=2) as xp,
                tc.tile_pool(name="rp", bufs=4) as rp,
                tc.tile_pool(name="qkps", bufs=2, space="PSUM") as qkps,
                tc.tile_pool(name="vps", bufs=2, space="PSUM") as vps,
            ):
                wqk_sb = qkvw.tile([128, HT, 384], dt.bfloat16)
                nc.sync.dma_start(wqk_sb[:], wqk[:])
                wv_sb = qkvw.tile([128, HT, 128], dt.bfloat16)
                cc_sb = qkvw.tile([128, T], dt.float32)
                ss_sb = qkvw.tile([128, T], dt.float32)
                nc.sync.dma_start(wv_sb[:], wv[:])
                nc.sync.dma_start(cc_sb[:], cc[:])
                nc.sync.dma_start(ss_sb[:], ss[:])
                nc.sync.dma_start(msk_sb[:], msk[:])
                nc.sync.dma_start(ones_sb[:], ones128[:])
                nc.sync.dma_start(wo_sb[:], wo[:])

                # q0,q1,k feature-major with weight-reuse: lhsT held over 4 t-tiles
                for th in range(2):                # halves of the 8 t-tiles
                    x_tiles = []
                    for i in range(4):
                        x_sb = xp.tile([128, HT, 512], dt.bfloat16, name=f"x{i}", bufs=1)
                        nc.sync.dma_start(
                            x_sb[:], xT[:, :, (th * 4 + i) * 512:(th * 4 + i + 1) * 512]
                        )
                        x_tiles.append(x_sb)
                    for ft in range(3):
                        pss = [
                            qkps.tile([128, 512], dt.float32, name=f"qk{i}", bufs=1)
                            for i in range(4)
                        ]
                        for ht in range(HT):
                            for i in range(4):
                                nc.tensor.matmul(
                                    pss[i][:],
                                    lhsT=wqk_sb[:, ht, ft * 128:(ft + 1) * 128],
                                    rhs=x_tiles[i][:, ht, :],
                                    start=(ht == 0),
                                    stop=(ht == HT - 1),
                                )
                        for i in range(4):
                            tt = th * 4 + i
                            tsl = bass.ts(tt, 512)
                            ps = pss[i]
                            sbq = rp.tile([128, 512], dt.float32)
                            nc.scalar.copy(sbq[:], ps[:])
                            tmp = rp.tile([128, 512], dt.float32)
                            nc.sync.dma_start(tmp[0:64, :], sbq[64:128, :])
                            nc.sync.dma_start(tmp[64:128, :], sbq[0:64, :])
                            qcc = rp.tile([128, 512], dt.float32)
                            nc.vector.tensor_tensor(qcc[:], sbq[:], cc_sb[:, tsl], mult)
                            qss = rp.tile([128, 512], dt.float32)
                            nc.vector.tensor_tensor(qss[:], tmp[:], ss_sb[:, tsl], mult)
                            if ft < 2:
                                dst = qcat[:, ft * T + tt * 512: ft * T + (tt + 1) * 512]
                            else:
                                dst = kT[:, tsl]
                            nc.vector.tensor_tensor(dst, qcc[:], qss[:], add)
                    # token-major v for this half
                    for i in range(4):
                        for st in range(4):
                            psv = vps.tile([128, 128], dt.float32)
                            for ht in range(HT):
                                nc.tensor.matmul(
                                    psv[:],
                                    lhsT=x_tiles[i][:, ht, st * 128:(st + 1) * 128],
                                    rhs=wv_sb[:, ht, :],
                                    start=(ht == 0),
                                    stop=(ht == HT - 1),
                                )
                            t128 = (th * 4 + i) * 4 + st
                            nc.vector.tensor_copy(vsb[:, t128 * 128:(t128 + 1) * 128], psv[:])

            # ---------------- attention + AllGather + W_o ----------------
            with (
                tc.tile_pool(name="spool", bufs=3, space="PSUM") as spool,
                tc.tile_pool(name="pvp", bufs=1, space="PSUM") as pvp,
                tc.tile_pool(name="denp", bufs=1, space="PSUM") as denp,
                tc.tile_pool(name="wops", bufs=1, space="PSUM") as wops,
                tc.tile_pool(name="probs", bufs=6) as probs,
                tc.tile_pool(name="smallp", bufs=4) as smallp,
                tc.tile_pool(name="ap", bufs=3) as apool,
                tc.tile_pool(name="wosb", bufs=2) as wosb,
                tc.tile_pool(name="outp", bufs=2) as outp,
            ):
                for j, qt in enumerate(QT_ORDER):
                    attn_chunk = dram.tile([256, 2 * 512], dt.bfloat16, name=f"attnc{j}")
                    for b in range(B):
                        col0 = b * 512
                        kts = 4 * qt + 4
                        ps_pv = [pvp.tile([128, 512], dt.float32, name=f"pv{hl}") for hl in range(2)]
                        ps_den = [denp.tile([128, 512], dt.float32, name=f"den{hl}") for hl in range(2)]
                        for kt in range(kts):
                            r = kt - 4 * qt
                            prob2 = []
                            for hl in range(2):
                                ps_s = spool.tile([128, 512], dt.float32)
                                nc.tensor.matmul(
                                    ps_s[:],
                                    lhsT=kT[:, b * S + kt * 128: b * S + (kt + 1) * 128],
                                    rhs=qcat[:, hl * T + b * S + qt * 512: hl * T + b * S + (qt + 1) * 512],
                                    start=True,
                                    stop=True,
                                )
                                prob = probs.tile([128, 512], dt.bfloat16)
                                if r >= 0:
                                    stg = probs.tile([128, 512], dt.bfloat16, name="stg")
                                    nc.scalar.activation(stg[:], ps_s[:], Exp, scale=SCALE)
                                    nc.vector.tensor_tensor(
                                        prob[:], stg[:], msk_sb[:, r * 512:(r + 1) * 512], mult
                                    )
                                else:
                                    nc.scalar.activation(prob[:], ps_s[:], Exp, scale=SCALE)
                                prob2.append(prob)
                            for hl in range(2):
                                nc.tensor.matmul(
                                    ps_pv[hl][:],
                                    lhsT=vsb[:, (b * HT + kt) * 128: (b * HT + kt + 1) * 128],
                                    rhs=prob2[hl][:],
                                    start=(kt == 0), stop=(kt == kts - 1),
                                )
                            for hl in range(2):
                                nc.tensor.matmul(
                                    ps_den[hl][:], lhsT=ones_sb[:], rhs=prob2[hl][:],
                                    start=(kt == 0), stop=(kt == kts - 1),
                                )
                        for hl in range(2):
                            den_sb = smallp.tile([128, 512], dt.float32)
                            nc.vector.reciprocal(den_sb[:], ps_den[hl][:])
                            attn_sb = apool.tile([128, 512], dt.bfloat16)
                            nc.vector.tensor_tensor(attn_sb[:], ps_pv[hl][:], den_sb[:], mult)
                            nc.sync.dma_start(
                                attn_chunk[hl * 128:(hl + 1) * 128, col0:col0 + 512],
                                attn_sb[:],
                            )
                    ag_out = dram.tile(
                        [HT, 128, 2 * 512], dt.bfloat16,
                        addr_space="Shared", name=f"agout{j}",
                    )
                    nc.gpsimd.collective_compute(
                        "AllGather", mybir.AluOpType.bypass, replica_groups=RG8,
                        ins=[attn_chunk.opt()], outs=[ag_out.opt()],
                    )
                    # W_o for this chunk's tokens (cb = batch)
                    for cb in range(2):
                        asb = wosb.tile([128, HT, 512], dt.bfloat16)
                        for dtt in range(HT):
                            nc.sync.dma_start(
                                asb[:, dtt, :], ag_out[dtt, :, cb * 512:(cb + 1) * 512]
                            )
                        for ct in range(2):
                            ps_o = wops.tile([128, 512], dt.float32)
                            for dtt in range(HT):
                                nc.tensor.matmul(
                                    ps_o[:],
                                    lhsT=wo_sb[:, dtt, ct * 128:(ct + 1) * 128],
                                    rhs=asb[:, dtt, :],
                                    start=(dtt == 0), stop=(dtt == HT - 1),
                                )
                            o_sb = outp.tile([128, 512], dt.float32)
                            nc.scalar.copy(o_sb[:], ps_o[:])
                            nc.sync.dma_start(
                                outT[ct * 128:(ct + 1) * 128, cb * S + qt * 512: cb * S + (qt + 1) * 512],
                                o_sb[:],
                            )
    nc.compile()
    return nc


# host-side input prep ------------------------------------------------------

_PERM = np.concatenate([np.arange(0, HD, 2), np.arange(1, HD, 2)])


def _rope_tables():
    freq = 1.0 / (10000.0 ** (np.arange(0, HD, 2, dtype=np.float64) / HD))
    pos = np.arange(S, dtype=np.float64)
    ang = np.outer(pos, freq)                       # [S, 64]
    cos = np.cos(ang).T.astype(np.float32)          # [64, S]
    sin = np.sin(ang).T.astype(np.float32)
    cc1 = np.concatenate([cos, cos], 0)             # [128, S]
    ss1 = np.concatenate([-sin, sin], 0)            # [128, S]
    return np.tile(cc1, (1, B)), np.tile(ss1, (1, B))   # [128, 4096]


def _prep_inputs(x, W_qkv, W_o):
    x = np.asarray(x, dtype=np.float32)
    W_qkv = np.asarray(W_qkv, dtype=np.float32)
    W_o = np.asarray(W_o, dtype=np.float32)

    xx = np.concatenate([x[0], x[1]], axis=0)       # [4096, 2048]
    xTd = np.ascontiguousarray(
        xx.T.reshape(HT, 128, T).transpose(1, 0, 2)
    ).astype(BF16)                                   # [128, HT, 4096]

    cc, ss = _rope_tables()

    mask = np.zeros((128, 4 * 512), dtype=np.float32)
    ii = np.arange(128)[:, None]
    jj = np.arange(512)[None, :]
    for r in range(4):
        mask[:, r * 512:(r + 1) * 512] = (jj >= ii + 128 * r)
    mask = mask.astype(BF16)

    ones128 = np.ones((128, 128), dtype=np.float32).astype(BF16)

    in_maps = []
    for c in range(8):
        kh = c // 2
        qr = W_qkv[256 * c: 256 * (c + 1)]           # rows of q heads 2c,2c+1
        qr = qr.reshape(2, HD, HIDDEN)[:, _PERM, :].reshape(256, HIDDEN)
        kr = W_qkv[HIDDEN + 128 * kh: HIDDEN + 128 * (kh + 1)][_PERM, :]
        vr = W_qkv[HIDDEN + 512 + 128 * kh: HIDDEN + 512 + 128 * (kh + 1)]
        wqkT = np.ascontiguousarray(
            np.concatenate([qr, kr], 0).T.reshape(HT, 128, 384).transpose(1, 0, 2)
        ).astype(BF16)                               # [128, HT, 384]
        wvT = np.ascontiguousarray(
            vr.T.reshape(HT, 128, 128).transpose(1, 0, 2)
        ).astype(BF16)
        woT = np.ascontiguousarray(
            W_o[256 * c: 256 * (c + 1)].T.reshape(HT, 128, 256).transpose(1, 0, 2)
        ).astype(BF16)
        in_maps.append({
            "xT": xTd, "wqk": wqkT, "wv": wvT, "wo": woT,
            "cc": cc, "ss": ss, "msk": mask, "ones128": ones128,
        })
    return in_maps


def kernel(x, W_qkv, W_o):
    global _COMPILED
    if _COMPILED is None:
        _COMPILED = _build()
    nc = _COMPILED
    in_maps = _prep_inputs(x, W_qkv, W_o)
    res = run_bass_kernel_spmd(nc, in_maps, list(range(8)))
    out = np.empty((B, S, HIDDEN), dtype=np.float32)
    for c in range(8):
        oT = res.results[c]["outT"]                  # [256, 4096]
        out[:, :, 256 * c: 256 * (c + 1)] = oT.reshape(256, B, S).transpose(1, 2, 0)
    return out


# revision 11
# speedup vs baseline: 1.2512x; 1.0610x over previous
"""Trainium2 Bass kernel for nn_Attention_4183298146960.

GQA causal attention layer: B=2, S=2048, HIDDEN=2048, 16 q heads / 4 kv heads,
head_dim=128, RoPE (interleaved pairs), causal softmax, output projection.

Sharding (8 cores, SPMD-uniform program):
  core c owns q heads {2c, 2c+1} and kv head c//2, for BOTH batches
  (tokens axis = [batch0 | batch1] = 4096).  QKV + RoPE + attention are
  fully local; the output projection needs all 16 heads' features, so the
  per-core attention outputs [256 feat, 4096 tok] are AllGathered (8-core
  mesh, chunked over q-tile slabs processed in reverse causal order so the
  gathers overlap attention compute), after which each core computes its
  256 output columns of W_o.

Layouts on device (partition dim first):
  feature-major qT/kT [head_dim, tokens] for scores; token-major v
  [tokens, head_dim] for PV; scores computed transposed [k, q] so softmax
  needs no max-subtraction (scores are O(+-10), exp is fp32-safe) and the
  denominator is a ones-matrix matmul producing the broadcast denominator
  directly; probabilities stay unnormalized until after PV.
  RoPE head dims are permuted [even | odd] via host-side W row permutation
  so the rotation is a 64-partition swap (SBUF->SBUF DMA) + DVE ops.
"""

import numpy as np
import ml_dtypes

import concourse.bass as bass
import concourse.mybir as mybir
import concourse.tile as tile
from concourse import bacc
from concourse.bass_utils import run_bass_kernel_spmd

BF16 = ml_dtypes.bfloat16

HEADS = 16
KV_HEADS = 4
HIDDEN = 2048
HD = 128
S = 2048
B = 2
T = B * S                      # 4096 token axis (both batches)
HT = HIDDEN // 128             # 16 hidden tiles
NQT = S // 512                 # 4 q-tiles of 512 per batch
SCALE = 1.0 / float(np.sqrt(HD))
RG8 = [[0, 1, 2, 3, 4, 5, 6, 7]]
QT_ORDER = [3, 2, 1, 0]        # chunk order: biggest (most causal kt) first

_COMPILED = None


def _build():
    dt = mybir.dt
    nc = bacc.Bacc("TRN2", target_bir_lowering=False, debug=False, num_devices=8)

    xT = nc.dram_tensor("xT", [128, HT, T], dt.bfloat16, kind="ExternalInput")
    wqk = nc.dram_tensor("wqk", [128, HT, 384], dt.bfloat16, kind="ExternalInput")
    wv = nc.dram_tensor("wv", [128, HT, 128], dt.bfloat16, kind="ExternalInput")
    wo = nc.dram_tensor("wo", [128, HT, 256], dt.bfloat16, kind="ExternalInput")
    cc = nc.dram_tensor("cc", [128, T], dt.float32, kind="ExternalInput")
    ss = nc.dram_tensor("ss", [128, T], dt.float32, kind="ExternalInput")
    msk = nc.dram_tensor("msk", [128, 4 * 512], dt.bfloat16, kind="ExternalInput")
    ones128 = nc.dram_tensor("ones128", [128, 128], dt.float32, kind="ExternalInput")
    outT = nc.dram_tensor("outT", [256, T], dt.float32, kind="ExternalOutput")

    mult = mybir.AluOpType.mult
    add = mybir.AluOpType.add
    divide = mybir.AluOpType.divide
    Exp = mybir.ActivationFunctionType.Exp

    with tile.TileContext(nc) as tc:
        with (
            tc.tile_pool(name="const", bufs=1) as constp,
            tc.tile_pool(name="dram", bufs=1, space="DRAM") as dram,
        ):
            qcat = constp.tile([128, 2 * T], dt.bfloat16)   # 2 local q heads
            kT = constp.tile([128, T], dt.bfloat16)
            vsb = constp.tile([128, T], dt.bfloat16)        # token-major v tiles
            wo_sb = constp.tile([128, HT, 256], dt.bfloat16)
            msk_sb = constp.tile([128, 4 * 512], dt.bfloat16)
            ones_sb = constp.tile([128, 128], dt.float32)

            # ---------------- QKV + RoPE ----------------
            with (
                tc.tile_pool(name="qkvw", bufs=1) as qkvw,
                tc.tile_pool(name="xp", bufs=2) as xp,
                tc.tile_pool(name="rp", bufs=4) as rp,
                tc.tile_pool(name="qkps", bufs=2, space="PSUM") as qkps,
                tc.tile_pool(name="vps", bufs=2, space="PSUM") as vps,
            ):
                wqk_sb = qkvw.tile([128, HT, 384], dt.bfloat16)
                nc.sync.dma_start(wqk_sb[:], wqk[:])
                wv_sb = qkvw.tile([128, HT, 128], dt.bfloat16)
                cc_sb = qkvw.tile([128, T], dt.float32)
                ss_sb = qkvw.tile([128, T], dt.float32)
                nc.sync.dma_start(wv_sb[:], wv[:])
                nc.sync.dma_start(cc_sb[:], cc[:])
                nc.sync.dma_start(ss_sb[:], ss[:])
                nc.sync.dma_start(msk_sb[:], msk[:])
                nc.sync.dma_start(ones_sb[:], ones128[:])
                nc.sync.dma_start(wo_sb[:], wo[:])

                # q0,q1,k feature-major with weight-reuse: lhsT held over 4 t-tiles
                for th in range(2):                # halves of the 8 t-tiles
                    x_tiles = []
                    for i in range(4):
                        x_sb = xp.tile([128, HT, 512], dt.bfloat16, name=f"x{i}", bufs=1)
                        for hq in range(4):  # split so MMs can start on early ht tiles
                            nc.sync.dma_start(
                                x_sb[:, hq * 4:(hq + 1) * 4, :],
                                xT[:, hq * 4:(hq + 1) * 4,
                                   (th * 4 + i) * 512:(th * 4 + i + 1) * 512],
                            )
                        x_tiles.append(x_sb)
                    for ft in range(3):
                        pss = [
                            qkps.tile([128, 512], dt.float32, name=f"qk{i}", bufs=1)
                            for i in range(4)
                        ]
                        for ht in range(HT):
                            for i in range(4):
                                nc.tensor.matmul(
                                    pss[i][:],
                                    lhsT=wqk_sb[:, ht, ft * 128:(ft + 1) * 128],
                                    rhs=x_tiles[i][:, ht, :],
                                    start=(ht == 0),
                                    stop=(ht == HT - 1),
                                )
                        for i in range(4):
                            tt = th * 4 + i
                            tsl = bass.ts(tt, 512)
                            ps = pss[i]
                            sbq = rp.tile([128, 512], dt.float32)
                            nc.scalar.copy(sbq[:], ps[:])
                            tmp = rp.tile([128, 512], dt.float32)
                            nc.sync.dma_start(tmp[0:64, :], sbq[64:128, :])
                            nc.sync.dma_start(tmp[64:128, :], sbq[0:64, :])
                            qcc = rp.tile([128, 512], dt.float32)
                            nc.vector.tensor_tensor(qcc[:], sbq[:], cc_sb[:, tsl], mult)
                            qss = rp.tile([128, 512], dt.float32)
                            nc.vector.tensor_tensor(qss[:], tmp[:], ss_sb[:, tsl], mult)
                            if ft < 2:
                                dst = qcat[:, ft * T + tt * 512: ft * T + (tt + 1) * 512]
                            else:
                                dst = kT[:, tsl]
                            nc.vector.tensor_tensor(dst, qcc[:], qss[:], add)
                    # token-major v for this half
                    for i in range(4):
                        for st in range(4):
                            psv = vps.tile([128, 128], dt.float32)
                            for ht in range(HT):
                                nc.tensor.matmul(
                                    psv[:],
                                    lhsT=x_tiles[i][:, ht, st * 128:(st + 1) * 128],
                                    rhs=wv_sb[:, ht, :],
                                    start=(ht == 0),
                                    stop=(ht == HT - 1),
                                )
                            t128 = (th * 4 + i) * 4 + st
                            nc.vector.tensor_copy(vsb[:, t128 * 128:(t128 + 1) * 128], psv[:])

            # ---------------- attention + AllGather + W_o ----------------
            with (
                tc.tile_pool(name="spool", bufs=4, space="PSUM") as spool,
                tc.tile_pool(name="pvp", bufs=1, space="PSUM") as pvp,
                tc.tile_pool(name="denp", bufs=1, space="PSUM") as denp,
                tc.tile_pool(name="wops", bufs=1, space="PSUM") as wops,
                tc.tile_pool(name="probs", bufs=6) as probs,
                tc.tile_pool(name="smallp", bufs=4) as smallp,
                tc.tile_pool(name="ap", bufs=3) as apool,
                tc.tile_pool(name="wosb", bufs=2) as wosb,
                tc.tile_pool(name="outp", bufs=2) as outp,
            ):
                for j, qt in enumerate(QT_ORDER):
                    attn_chunk = dram.tile([256, 2 * 512], dt.bfloat16, name=f"attnc{j}")
                    for b in range(B):
                        col0 = b * 512
                        kts = 4 * qt + 4
                        ps_pv = [pvp.tile([128, 512], dt.float32, name=f"pv{hl}") for hl in range(2)]
                        acc = [smallp.tile([128, 512], dt.float32, name=f"acc{hl}", bufs=2) for hl in range(2)]
                        for kt in range(kts):
                            r = kt - 4 * qt
                            prob2 = []
                            for hl in range(2):
                                ps_s = spool.tile([128, 512], dt.float32)
                                nc.tensor.matmul(
                                    ps_s[:],
                                    lhsT=kT[:, b * S + kt * 128: b * S + (kt + 1) * 128],
                                    rhs=qcat[:, hl * T + b * S + qt * 512: hl * T + b * S + (qt + 1) * 512],
                                    start=True,
                                    stop=True,
                                )
                                prob = probs.tile([128, 512], dt.bfloat16)
                                if r >= 0:
                                    stg = probs.tile([128, 512], dt.bfloat16, name="stg")
                                    nc.scalar.activation(stg[:], ps_s[:], Exp, scale=SCALE)
                                    nc.vector.tensor_tensor(
                                        prob[:], stg[:], msk_sb[:, r * 512:(r + 1) * 512], mult
                                    )
                                else:
                                    nc.scalar.activation(prob[:], ps_s[:], Exp, scale=SCALE)
                                prob2.append(prob)
                            for hl in range(2):
                                nc.tensor.matmul(
                                    ps_pv[hl][:],
                                    lhsT=vsb[:, (b * HT + kt) * 128: (b * HT + kt + 1) * 128],
                                    rhs=prob2[hl][:],
                                    start=(kt == 0), stop=(kt == kts - 1),
                                )
                            for hl in range(2):
                                # denominator partial sums on DVE (sum over k-tiles)
                                if kt == 0:
                                    nc.vector.tensor_copy(acc[hl][:], prob2[hl][:])
                                else:
                                    nc.vector.tensor_tensor(
                                        acc[hl][:], acc[hl][:], prob2[hl][:], add
                                    )
                        for hl in range(2):
                            # partition-reduce + broadcast denominators in one matmul
                            ps_den = denp.tile([128, 512], dt.float32)
                            nc.tensor.matmul(
                                ps_den[:], lhsT=ones_sb[:], rhs=acc[hl][:],
                                start=True, stop=True,
                            )
                            den_sb = smallp.tile([128, 512], dt.float32)
                            nc.vector.reciprocal(den_sb[:], ps_den[:])
                            attn_sb = apool.tile([128, 512], dt.bfloat16)
                            nc.vector.tensor_tensor(attn_sb[:], ps_pv[hl][:], den_sb[:], mult)
                            nc.sync.dma_start(
                                attn_chunk[hl * 128:(hl + 1) * 128, col0:col0 + 512],
                                attn_sb[:],
                            )
                    ag_out = dram.tile(
                        [HT, 128, 2 * 512], dt.bfloat16,
                        addr_space="Shared", name=f"agout{j}",
                    )
                    nc.gpsimd.collective_compute(
                        "AllGather", mybir.AluOpType.bypass, replica_groups=RG8,
                        ins=[attn_chunk.opt()], outs=[ag_out.opt()],
                    )
                    # W_o for this chunk's tokens (cb = batch)
                    for cb in range(2):
                        asb = wosb.tile([128, HT, 512], dt.bfloat16)
                        for dtt in range(HT):
                            nc.sync.dma_start(
                                asb[:, dtt, :], ag_out[dtt, :, cb * 512:(cb + 1) * 512]
                            )
                        for ct in range(2):
                            ps_o = wops.tile([128, 512], dt.float32)
                            for dtt in range(HT):
                                nc.tensor.matmul(
                                    ps_o[:],
                                    lhsT=wo_sb[:, dtt, ct * 128:(ct + 1) * 128],
                                    rhs=asb[:, dtt, :],
                                    start=(dtt == 0), stop=(dtt == HT - 1),
                                )
                            o_sb = outp.tile([128, 512], dt.float32)
                            nc.scalar.copy(o_sb[:], ps_o[:])
                            nc.sync.dma_start(
                                outT[ct * 128:(ct + 1) * 128, cb * S + qt * 512: cb * S + (qt + 1) * 512],
                                o_sb[:],
                            )
    nc.compile()
    return nc


# host-side input prep ------------------------------------------------------

_PERM = np.concatenate([np.arange(0, HD, 2), np.arange(1, HD, 2)])


def _rope_tables():
    freq = 1.0 / (10000.0 ** (np.arange(0, HD, 2, dtype=np.float64) / HD))
    pos = np.arange(S, dtype=np.float64)
    ang = np.outer(pos, freq)                       # [S, 64]
    cos = np.cos(ang).T.astype(np.float32)          # [64, S]
    sin = np.sin(ang).T.astype(np.float32)
    cc1 = np.concatenate([cos, cos], 0)             # [128, S]
    ss1 = np.concatenate([-sin, sin], 0)            # [128, S]
    return np.tile(cc1, (1, B)), np.tile(ss1, (1, B))   # [128, 4096]


def _prep_inputs(x, W_qkv, W_o):
    x = np.asarray(x, dtype=np.float32)
    W_qkv = np.asarray(W_qkv, dtype=np.float32)
    W_o = np.asarray(W_o, dtype=np.float32)

    xx = np.concatenate([x[0], x[1]], axis=0)       # [4096, 2048]
    xTd = np.ascontiguousarray(
        xx.T.reshape(HT, 128, T).transpose(1, 0, 2)
    ).astype(BF16)                                   # [128, HT, 4096]

    cc, ss = _rope_tables()

    mask = np.zeros((128, 4 * 512), dtype=np.float32)
    ii = np.arange(128)[:, None]
    jj = np.arange(512)[None, :]
    for r in range(4):
        mask[:, r * 512:(r + 1) * 512] = (jj >= ii + 128 * r)
    mask = mask.astype(BF16)

    ones128 = np.ones((128, 128), dtype=np.float32)

    in_maps = []
    for c in range(8):
        kh = c // 2
        qr = W_qkv[256 * c: 256 * (c + 1)]           # rows of q heads 2c,2c+1
        qr = qr.reshape(2, HD, HIDDEN)[:, _PERM, :].reshape(256, HIDDEN)
        kr = W_qkv[HIDDEN + 128 * kh: HIDDEN + 128 * (kh + 1)][_PERM, :]
        vr = W_qkv[HIDDEN + 512 + 128 * kh: HIDDEN + 512 + 128 * (kh + 1)]
        wqkT = np.ascontiguousarray(
            np.concatenate([qr, kr], 0).T.reshape(HT, 128, 384).transpose(1, 0, 2)
        ).astype(BF16)                               # [128, HT, 384]
        wvT = np.ascontiguousarray(
            vr.T.reshape(HT, 128, 128).transpose(1, 0, 2)
        ).astype(BF16)
        woT = np.ascontiguousarray(
            W_o[256 * c: 256 * (c + 1)].T.reshape(HT, 128, 256).transpose(1, 0, 2)
        ).astype(BF16)
        in_maps.append({
            "xT": xTd, "wqk": wqkT, "wv": wvT, "wo": woT,
            "cc": cc, "ss": ss, "msk": mask, "ones128": ones128,
        })
    return in_maps


def kernel(x, W_qkv, W_o):
    global _COMPILED
    if _COMPILED is None:
        _COMPILED = _build()
    nc = _COMPILED
    in_maps = _prep_inputs(x, W_qkv, W_o)
    res = run_bass_kernel_spmd(nc, in_maps, list(range(8)))
    out = np.empty((B, S, HIDDEN), dtype=np.float32)
    for c in range(8):
        oT = res.results[c]["outT"]                  # [256, 4096]
        out[:, :, 256 * c: 256 * (c + 1)] = oT.reshape(256, B, S).transpose(1, 2, 0)
    return out
